# revision 21
# baseline (speedup 1.0000x reference)
"""Trainium2 Bass kernel for a 16-layer fully-connected chain (matvec per layer).

Computation (reference):
    v = x                       # [2048]
    for i in 0..13:  v = silu(W[i] @ v + b[i])
    out = W[14] @ v + b[14]

Strategy (8 NeuronCores):
  - Weights are stored/streamed in bf16 (half the HBM traffic of f32).
  - Activations are carried as a bf16 hi/lo pair (v = hi + lo), which keeps
    the effective activation precision ~fp24 while still feeding the bf16
    tensor engine; both halves ride the same stationary weight tile as two
    moving columns, so the extra cost is ~1 PE column per tile. Total chain
    rel-err ~1.2e-2 (bf16-weight-limited) vs the 2e-2 gate.
  - Layers alternate ROW-sharded and COLUMN-sharded so only ONE collective
    per TWO layers is needed (7 AllGathers total instead of 14):
      * row layer (even i): core c computes its 256 output neurons from the
        full activation vector (which every core holds). No collective.
      * col layer (odd i): core c multiplies its 256 local activations by
        the matching 256-column slice of W[i], producing a partial sum over
        ALL 2048 outputs. The partials are AllGathered (8 x 8KB f32) and
        each core reduces them locally (one DVE tensor_reduce over the 8
        rank slots plus a pre-filled bias slot), applies silu -- after
        which every core again holds the full activation vector.
  - All matmuls are weight-stationary (lhsT = weight tile [128k x 128m],
    rhs = activation hi/lo pair [128, 2]) so activations live on 128
    partitions end-to-end; no transposes are needed anywhere.
  - Biases are f32: row-layer bias rides the hi+lo PSUM combine
    (scalar_tensor_tensor); col-layer bias is slot 9 of the reduce.
  - Weight layout is prepared host-side so each layer's per-core slice is a
    single contiguous 1 MB HBM->SBUF DMA ([128, 4096] bf16); the weight
    pool holds all 15 layer tiles (15 x 8KB/partition) so the Tile
    scheduler is free to stream weights ahead of the serial exchange chain.

TimelineSim cost-model exec: ~195 us (baseline row-sharded f32 kernel with
14 AllGathers: ~466 us). Measured rel err vs the f32 reference: 1.17e-2.

Neuron-index conventions (baked into the host-side permutations):
  full vector:  sigma(p, t) = p*16 + t     (p = SBUF partition, t = column)
  local 256:    lam(c, m)   = c*128 + m    (c = psum column, m = partition)
"""

import numpy as np

_L = 15          # weight matrices
_N = 2048        # neurons per layer
_M = 8           # cores
_SH = _N // _M   # 256 local slice
_P = 128
_KT = _N // _P   # 16 k-chunks per full vector
_NR = 8          # row layers: 0, 2, ..., 14
_NC = 7          # col layers: 1, 3, ..., 13

_CACHE = {}


def _build():
    import concourse.bacc as bacc
    import concourse.mybir as mybir
    import concourse.tile as tile

    f32 = mybir.dt.float32
    bf16 = mybir.dt.bfloat16
    silu = mybir.ActivationFunctionType.Silu
    add = mybir.AluOpType.add
    sub = mybir.AluOpType.subtract

    nc = bacc.Bacc("TRN2", target_bir_lowering=False, debug=False,
                   num_devices=_M)

    # [8 row layers][p][ (kt, c, m) ] -- lhsT tile (kt,c) at free offset
    # (kt*2+c)*128;  element [p, m] = W[i][c*128+m + core*256, sigma(p,kt)]
    wrow = nc.dram_tensor("wrow", [_NR, _P, 2 * _KT * _P], bf16,
                          kind="ExternalInput")
    # [7 col layers][p][ (c, mt, m) ] -- lhsT tile (c,mt) at free offset
    # (c*16+mt)*128;  element [p, m] = W[i][m*16+mt, c*128+p + core*256]
    wcol = nc.dram_tensor("wcol", [_NC, _P, 2 * _KT * _P], bf16,
                          kind="ExternalInput")
    # row biases [p, (i8, c)] f32: value b[i][core*256 + c*128 + p]
    brow = nc.dram_tensor("brow", [_P, _NR * 2], f32, kind="ExternalInput")
    # col biases [p, (i7, t)] f32: value b[i][sigma(p,t)]
    bcol = nc.dram_tensor("bcol", [_P, _NC * _KT], f32, kind="ExternalInput")
    # x in full-vector layout, hi/lo split: [p, (h, t)] bf16
    x0 = nc.dram_tensor("x0", [_P, 2 * _KT], bf16, kind="ExternalInput")
    # last row layer output [p, c] f32 (global j = core*256 + c*128 + p)
    out = nc.dram_tensor("out", [_P, 2], f32, kind="ExternalOutput")

    with tile.TileContext(nc) as tc:
        with (
            tc.tile_pool(name="w", bufs=15) as wpool,
            tc.tile_pool(name="v", bufs=3) as vpool,
            tc.tile_pool(name="s", bufs=4) as spool,
            tc.tile_pool(name="r", bufs=4) as rpool,
            tc.tile_pool(name="consts", bufs=1) as cpool,
            tc.tile_pool(name="psr", bufs=4, space="PSUM") as psrpool,
            tc.tile_pool(name="psc", bufs=3, space="PSUM") as pscpool,
            tc.tile_pool(name="dram", bufs=2, space="DRAM") as dpool,
        ):
            brow_t = cpool.tile([_P, _NR * 2], f32)
            nc.sync.dma_start(brow_t[:], brow.ap())
            bcol_t = cpool.tile([_P, _NC * _KT], f32)
            nc.sync.dma_start(bcol_t[:], bcol.ap())

            # persistent landing area: per pair, 8 rank slots + bias slot 9
            # (slot 9 pre-filled once so the cross-core reduce covers bias)
            _SL = (_M + 1) * _KT  # 144 floats per pair
            landall = cpool.tile([_P, _NC * _SL], f32)
            nc.vector.tensor_copy(
                landall[:].rearrange("p (j s) -> p j s", j=_NC)[:, :, _M * _KT:],
                bcol_t[:].rearrange("p (j t) -> p j t", j=_NC))

            v = vpool.tile([_P, 2 * _KT], bf16, tag="v")
            nc.sync.dma_start(v[:], x0.ap())

            def row_layer(i8, v):
                """v [128, (h,t)] bf16 -> pre [128, 2] f32 (bias added)."""
                w = wpool.tile([_P, 2 * _KT * _P], bf16, tag="w")
                nc.sync.dma_start(w[:], wrow.ap()[i8])
                ps = psrpool.tile([_P, 4], f32, tag="psr")
                for c in range(2):
                    for kt in range(_KT):
                        f = (kt * 2 + c) * _P
                        nc.tensor.matmul(
                            ps[:, 2 * c:2 * c + 2],
                            lhsT=w[:, f:f + _P],
                            rhs=v[:, kt:kt + 17:16],
                            start=(kt == 0),
                            stop=(kt == _KT - 1),
                        )
                # only one vector-op input may be PSUM: stage lo in SBUF
                lo_s = rpool.tile([_P, 2], f32, tag="rlo")
                nc.vector.tensor_copy(lo_s[:], ps[:, 1:4:2])
                pre = rpool.tile([_P, 2], f32, tag="prer")
                for c in range(2):
                    nc.vector.scalar_tensor_tensor(
                        pre[:, c:c + 1],
                        ps[:, 2 * c:2 * c + 1],
                        brow_t[:, i8 * 2 + c:i8 * 2 + c + 1],
                        lo_s[:, c:c + 1],
                        add, add,
                    )
                return pre

            for pair in range(_NC):
                # --- row layer 2*pair ---
                pre = row_layer(pair, v)
                s32 = spool.tile([_P, 2], f32, tag="s32")
                nc.scalar.activation(s32[:], pre[:], silu)
                s = spool.tile([_P, 4], bf16, tag="s")
                nc.vector.tensor_copy(s[:, 0:2], s32[:])
                nc.vector.tensor_tensor(s[:, 2:4], s32[:], s[:, 0:2], sub)

                # --- col layer 2*pair+1: partial over all 2048 outputs ---
                w = wpool.tile([_P, 2 * _KT * _P], bf16, tag="w")
                nc.sync.dma_start(w[:], wcol.ap()[pair])
                pc = pscpool.tile([_P, 2 * _KT], f32, tag="psc")
                for mt in range(_KT):
                    for c in range(2):
                        f = (c * _KT + mt) * _P
                        nc.tensor.matmul(
                            pc[:, 2 * mt:2 * mt + 2],
                            lhsT=w[:, f:f + _P],
                            rhs=s[:, c:c + 3:2],
                            start=(c == 0),
                            stop=(c == 1),
                        )
                # combine hi+lo psum columns -> partial [128, 16] f32
                # (stage hi in SBUF first: one PSUM input per vector op)
                hi_s = spool.tile([_P, _KT], f32, tag="his")
                nc.vector.tensor_copy(hi_s[:], pc[:, 0:2 * _KT:2])
                sp = spool.tile([_P, _KT], f32, tag="sp")
                nc.vector.tensor_tensor(
                    sp[:], pc[:, 1:2 * _KT:2], hi_s[:], add)

                # --- exchange: AllGather partials, reduce locally ---
                cc_in = dpool.tile([_P, _KT], f32, tag="ccin")
                nc.sync.dma_start(cc_in[:], sp[:])
                cc_out = dpool.tile([_M, _N], f32, tag="ccout")
                nc.gpsimd.collective_compute(
                    "AllGather",
                    mybir.AluOpType.bypass,
                    replica_groups=[list(range(_M))],
                    ins=[cc_in.opt()],
                    outs=[cc_out.opt()],
                )
                land = landall[:, pair * _SL:pair * _SL + _M * _KT]
                nc.sync.dma_start(
                    land.rearrange("p (r t) -> p r t", r=_M),
                    cc_out[:, :].rearrange("r (p t) -> p r t", p=_P))
                # reduce over 8 rank slots + the pre-filled bias slot
                pre = rpool.tile([_P, _KT], f32, tag="pre")
                nc.vector.tensor_reduce(
                    pre[:],
                    landall[:, pair * _SL:(pair + 1) * _SL]
                    .rearrange("p (r t) -> p t r", r=_M + 1),
                    axis=mybir.AxisListType.X,
                    op=add,
                )
                v32 = rpool.tile([_P, _KT], f32, tag="v32")
                nc.scalar.activation(v32[:], pre[:], silu)
                v = vpool.tile([_P, 2 * _KT], bf16, tag="v")
                nc.vector.tensor_copy(v[:, 0:_KT], v32[:])
                nc.vector.tensor_tensor(v[:, _KT:2 * _KT], v32[:], v[:, 0:_KT], sub)

            # --- last row layer (identity activation, f32 out) ---
            pre = row_layer(_NR - 1, v)
            nc.sync.dma_start(out.ap(), pre[:])

    nc.compile()
    return nc


def _prep_inputs(x, W, b):
    import ml_dtypes
    bf16 = ml_dtypes.bfloat16
    W = np.ascontiguousarray(W, dtype=np.float32)
    b = np.ascontiguousarray(b, dtype=np.float32)
    x = np.ascontiguousarray(x, dtype=np.float32)

    row_ids = list(range(0, _L, 2))   # 0,2,...,14
    col_ids = list(range(1, _L, 2))   # 1,3,...,13

    # row weights: [i8][cc][p, (kt, c, m)],  elem = W[i][cc*256+c*128+m, p*16+kt]
    wrow = np.empty((_M, _NR, _P, 2 * _KT * _P), dtype=bf16)
    for n, i in enumerate(row_ids):
        Wi = W[i].reshape(_M, 2, _P, _P, _KT)       # [cc, c, m, p, t]
        Wi = Wi.transpose(0, 3, 4, 1, 2)            # [cc, p, t, c, m]
        wrow[:, n] = Wi.reshape(_M, _P, 2 * _KT * _P).astype(bf16)

    # col weights: [i7][cc][p, (c, mt, m)], elem = W[i][m*16+mt, cc*256+c*128+p]
    wcol = np.empty((_M, _NC, _P, 2 * _KT * _P), dtype=bf16)
    for n, i in enumerate(col_ids):
        Wi = W[i].reshape(_P, _KT, _M, 2, _P)       # [m, mt, cc, c, p]
        Wi = Wi.transpose(2, 4, 3, 1, 0)            # [cc, p, c, mt, m]
        wcol[:, n] = Wi.reshape(_M, _P, 2 * _KT * _P).astype(bf16)

    # row biases: [cc][p, (i8, c)] f32 = b[i][cc*256 + c*128 + p]
    brow = np.empty((_M, _P, _NR * 2), dtype=np.float32)
    for n, i in enumerate(row_ids):
        bi = b[i].reshape(_M, 2, _P)                # [cc, c, p]
        brow[:, :, 2 * n:2 * n + 2] = bi.transpose(0, 2, 1)

    # col biases (replicated): [p, (i7, t)] = b[i][p*16+t], f32
    bcol = np.empty((_P, _NC * _KT), dtype=np.float32)
    for n, i in enumerate(col_ids):
        bcol[:, n * _KT:(n + 1) * _KT] = b[i].reshape(_P, _KT)

    # x hi/lo split: [p, (h, t)] with value x[p*16+t]
    xm = x.reshape(_P, _KT)
    xhi = xm.astype(bf16)
    xlo = (xm - xhi.astype(np.float32)).astype(bf16)
    x0 = np.concatenate([xhi, xlo], axis=1)

    in_maps = []
    for c in range(_M):
        in_maps.append({
            "wrow": np.ascontiguousarray(wrow[c]),
            "wcol": np.ascontiguousarray(wcol[c]),
            "brow": np.ascontiguousarray(brow[c]),
            "bcol": bcol,
            "x0": x0,
        })
    return in_maps


def kernel(x, W, b, _trace=False):
    from concourse.bass_utils import run_bass_kernel_spmd

    key = "nc"
    if key not in _CACHE:
        _CACHE[key] = _build()
    nc = _CACHE[key]

    in_maps = _prep_inputs(x, W, b)
    res = run_bass_kernel_spmd(
        nc, in_maps, core_ids=list(range(_M)), trace=_trace)
    _CACHE["last_results"] = res
    # out[p, c] holds global neuron core*256 + c*128 + p
    return np.concatenate(
        [res.results[c]["out"].T.reshape(-1) for c in range(_M)])


# revision 33
# speedup vs baseline: 1.2649x; 1.2649x over previous
"""Trainium2 Bass kernel for a 16-layer fully-connected chain (matvec per layer).

Computation (reference):
    v = x                       # [2048]
    for i in 0..13:  v = silu(W[i] @ v + b[i])
    out = W[14] @ v + b[14]

Strategy (8 NeuronCores):
  - Weights stream as bf16 (half the HBM traffic of f32). Activations are
    carried as a bf16 hi/lo pair (v = hi + lo, ~fp24 effective) riding the
    same stationary weight tile as two moving columns. Chain rel-err
    ~1.2e-2 (bf16-weight-limited) vs the 2e-2 gate.
  - Cross-core exchanges are the dominant serial cost (a collective has a
    large flat latency), so the schedule trades extra -- mostly hidden --
    weight DMA for fewer collectives. 15 layers run as 5 exchange segments
    with a fully-replicated layer opening segments 2-5:

    L0 r, L1 c, AG | L2 r, L3 c, AG, L4 F | L5 r, L6 c, AG, L7 F |
    L8 r, L9 c, AG, L10 F | L11 r, L12 c, AG, L13 F | L14 r
      * row layer: core c computes its 256 output neurons from the full
        activation vector (1/8 of W, no exchange).
      * col layer: core c multiplies its 256 local activations by the
        matching 256-column slice of W (1/8), producing a partial sum over
        all 2048 outputs; partials are AllGathered (8 x 8KB f32) and each
        core reduces them locally (one DVE tensor_reduce over the 8 rank
        slots plus a pre-filled bias slot), then silu.
      * full layer: every core redundantly computes the whole layer from
        the full weight matrix (4 MB bf16 per core instead of 0.5 MB).
        The extra DMA hides under the exchange chain; the exchange it
        replaces does not. 5 collectives total instead of 7.
  - All matmuls are weight-stationary (lhsT = [128k x 128m] tile, rhs =
    hi/lo pair [128, 2]); activations stay on 128 partitions end-to-end.
  - PSUM hi/lo combines are single DVE tensor_reduce ops over a [p, n, 2]
    view; biases are f32 tensors added on DVE (col-layer bias rides the
    exchange reduce as landing slot 9).

TimelineSim cost-model exec: ~154 us (prior 7-AG version: ~195 us;
original baseline row-sharded f32 kernel with 14 AllGathers: ~466 us).

Neuron-index conventions (baked into the host-side permutations):
  full vector:  sigma(p, t) = p*16 + t     (p = SBUF partition, t = column)
  local 256:    lam(c, m)   = c*128 + m    (c = psum column, m = partition)
"""

import numpy as np

_L = 15          # weight matrices
_N = 2048        # neurons per layer
_M = 8           # cores
_SH = _N // _M   # 256 local slice
_P = 128
_KT = _N // _P   # 16 k-chunks per full vector
_NE = 5          # exchanges (AllGathers)
_FULL_AFTER = [1, 2, 3, 4]   # segments followed by a replicated full layer

def _schedule(ne, full_after):
    rows, cols, fulls = [], [], []
    li = 0
    for seg in range(ne):
        rows.append(li); li += 1
        cols.append(li); li += 1
        if seg in full_after:
            fulls.append(li); li += 1
    rows.append(li); li += 1
    assert li == _L, li
    return rows, cols, fulls

_ROW_IDS, _COL_IDS, _FULL_IDS = _schedule(_NE, _FULL_AFTER)
_NF = len(_FULL_IDS)
_NRW = len(_ROW_IDS)

_CACHE = {}


def _build():
    import concourse.bacc as bacc
    import concourse.mybir as mybir
    import concourse.tile as tile

    f32 = mybir.dt.float32
    bf16 = mybir.dt.bfloat16
    silu = mybir.ActivationFunctionType.Silu
    add = mybir.AluOpType.add
    sub = mybir.AluOpType.subtract

    nc = bacc.Bacc("TRN2", target_bir_lowering=False, debug=False,
                   num_devices=_M)

    # row layers [6][p][(kt, c, m)]: elem = W[i][core*256+c*128+m, p*16+kt]
    wrow = nc.dram_tensor("wrow", [_NRW, _P, 2 * _KT * _P], bf16,
                          kind="ExternalInput")
    # col layers [5][p][(c, mt, m)]: elem = W[i][m*16+mt, core*256+c*128+p]
    wcol = nc.dram_tensor("wcol", [_NE, _P, 2 * _KT * _P], bf16,
                          kind="ExternalInput")
    # full layers [4][p][(kt, mt, m)]: elem = W[i][m*16+mt, p*16+kt]
    wfull = nc.dram_tensor("wfull", [max(_NF, 1), _P, _KT * _KT * _P],
                           bf16, kind="ExternalInput")
    # row biases [p, (i6, c)] f32: value b[i][core*256 + c*128 + p]
    brow = nc.dram_tensor("brow", [_P, _NRW * 2], f32, kind="ExternalInput")
    # col biases [p, (i5, t)] f32: value b[i][sigma(p,t)]
    bcol = nc.dram_tensor("bcol", [_P, _NE * _KT], f32, kind="ExternalInput")
    # full biases [p, (i4, t)] f32: value b[i][sigma(p,t)]
    bfull = nc.dram_tensor("bfull", [_P, max(_NF, 1) * _KT], f32,
                           kind="ExternalInput")
    # x in full-vector layout, hi/lo split: [p, (h, t)] bf16
    x0 = nc.dram_tensor("x0", [_P, 2 * _KT], bf16, kind="ExternalInput")
    # last row layer output [p, c] f32 (global j = core*256 + c*128 + p)
    out = nc.dram_tensor("out", [_P, 2], f32, kind="ExternalOutput")

    with tile.TileContext(nc) as tc:
        with (
            tc.tile_pool(name="w", bufs=8) as wpool,
            tc.tile_pool(name="wf", bufs=2) as wfhpool,
            tc.tile_pool(name="v", bufs=3) as vpool,
            tc.tile_pool(name="s", bufs=4) as spool,
            tc.tile_pool(name="r", bufs=4) as rpool,
            tc.tile_pool(name="consts", bufs=1) as cpool,
            tc.tile_pool(name="psr", bufs=2, space="PSUM") as psrpool,
            tc.tile_pool(name="psc", bufs=2, space="PSUM") as pscpool,
            tc.tile_pool(name="dram", bufs=2, space="DRAM") as dpool,
        ):
            brow_t = cpool.tile([_P, _NRW * 2], f32)
            nc.sync.dma_start(brow_t[:], brow.ap())
            bcol_t = cpool.tile([_P, _NE * _KT], f32)
            nc.sync.dma_start(bcol_t[:], bcol.ap())
            bfull_t = cpool.tile([_P, _NF * _KT], f32)
            nc.sync.dma_start(bfull_t[:], bfull.ap())

            # persistent landing area: per exchange, 8 rank slots + bias
            # slot 9 (pre-filled once; the reduce then covers the bias)
            _SL = (_M + 1) * _KT  # 144 floats per exchange
            landall = cpool.tile([_P, _NE * _SL], f32)
            nc.vector.tensor_copy(
                landall[:].rearrange("p (j s) -> p j s", j=_NE)[:, :, _M * _KT:],
                bcol_t[:].rearrange("p (j t) -> p j t", j=_NE))

            v = vpool.tile([_P, 2 * _KT], bf16, tag="v")
            nc.sync.dma_start(v[:], x0.ap())

            def hi_lo_reduce(ps, n, tag):
                """PSUM [p, (n, h)] -> SBUF f32 [p, n] (hi+lo summed)."""
                sm = rpool.tile([_P, n], f32, tag=tag)
                nc.vector.tensor_reduce(
                    sm[:],
                    ps[:].rearrange("p (n h) -> p n h", h=2),
                    axis=mybir.AxisListType.X,
                    op=add,
                )
                return sm

            def silu_split(pre, n, tag):
                """f32 [p, n] -> bf16 [p, (h, n)] hi/lo pair of silu."""
                s32 = spool.tile([_P, n], f32, tag=tag + "32")
                nc.scalar.activation(s32[:], pre[:], silu)
                s = spool.tile([_P, 2 * n], bf16, tag=tag)
                nc.vector.tensor_copy(s[:, 0:n], s32[:])
                nc.vector.tensor_tensor(s[:, n:2 * n], s32[:], s[:, 0:n], sub)
                return s

            def row_layer(i6, v, w=None):
                """v [p,(h,t)] bf16 -> pre [p, 2] f32 (bias added)."""
                if w is None:
                    w = wpool.tile([_P, 2 * _KT * _P], bf16, tag="w")
                    nc.sync.dma_start(w[:], wrow.ap()[i6])
                ps = psrpool.tile([_P, 4], f32, tag="psr")
                for c in range(2):
                    for kt in range(_KT):
                        f = (kt * 2 + c) * _P
                        nc.tensor.matmul(
                            ps[:, 2 * c:2 * c + 2],
                            lhsT=w[:, f:f + _P],
                            rhs=v[:, kt:kt + 17:16],
                            start=(kt == 0),
                            stop=(kt == _KT - 1),
                        )
                sm = hi_lo_reduce(ps, 2, "rsum")
                pre = rpool.tile([_P, 2], f32, tag="prer")
                nc.vector.tensor_tensor(
                    pre[:], sm[:], brow_t[:, i6 * 2:i6 * 2 + 2], add)
                return pre

            for seg in range(_NE):
                # --- row layer ---
                pre = row_layer(seg, v)
                s = silu_split(pre, 2, "s")

                # --- col layer: partial over all 2048 outputs ---
                w = wpool.tile([_P, 2 * _KT * _P], bf16, tag="w")
                nc.sync.dma_start(w[:], wcol.ap()[seg])
                pc = pscpool.tile([_P, 2 * _KT], f32, tag="psc")
                for mt in range(_KT):
                    for c in range(2):
                        f = (c * _KT + mt) * _P
                        nc.tensor.matmul(
                            pc[:, 2 * mt:2 * mt + 2],
                            lhsT=w[:, f:f + _P],
                            rhs=s[:, c:c + 3:2],
                            start=(c == 0),
                            stop=(c == 1),
                        )
                sp = hi_lo_reduce(pc, _KT, "sp")

                # --- exchange: AllGather partials, reduce + bias + silu ---
                cc_in = dpool.tile([_P, _KT], f32, tag="ccin")
                nc.sync.dma_start(cc_in[:], sp[:])
                cc_out = dpool.tile([_M, _N], f32, tag="ccout")
                nc.gpsimd.collective_compute(
                    "AllGather",
                    mybir.AluOpType.bypass,
                    replica_groups=[list(range(_M))],
                    ins=[cc_in.opt()],
                    outs=[cc_out.opt()],
                )
                land = landall[:, seg * _SL:seg * _SL + _M * _KT]
                nc.sync.dma_start(
                    land.rearrange("p (r t) -> p r t", r=_M),
                    cc_out[:, :].rearrange("r (p t) -> p r t", p=_P))
                pre_v = rpool.tile([_P, _KT], f32, tag="prev")
                nc.vector.tensor_reduce(
                    pre_v[:],
                    landall[:, seg * _SL:(seg + 1) * _SL]
                    .rearrange("p (r t) -> p t r", r=_M + 1),
                    axis=mybir.AxisListType.X,
                    op=add,
                )
                v = silu_split(pre_v, _KT, "v")

                # --- full layer (replicated on every core) ---
                if seg in _FULL_AFTER:
                    fi = _FULL_AFTER.index(seg)
                    wf = wfhpool.tile([_P, _KT * _KT * _P], bf16, tag="wf")
                    # split the 4MB load into 4 chunks so a latency-critical
                    # exchange bounce DMA never queues behind more than ~6us
                    # of weight transfer on the SP DMA path
                    _ck = _KT * _KT * _P // 4
                    for _c in range(4):
                        nc.sync.dma_start(
                            wf[:, _c * _ck:(_c + 1) * _ck],
                            wfull.ap()[fi][:, _c * _ck:(_c + 1) * _ck])
                    pf = pscpool.tile([_P, 2 * _KT], f32, tag="psf")
                    for mt in range(_KT):
                        for kt in range(_KT):
                            f = (kt * _KT + mt) * _P
                            nc.tensor.matmul(
                                pf[:, 2 * mt:2 * mt + 2],
                                lhsT=wf[:, f:f + _P],
                                rhs=v[:, kt:kt + 17:16],
                                start=(kt == 0),
                                stop=(kt == _KT - 1),
                            )
                    sf = hi_lo_reduce(pf, _KT, "sf")
                    pre_f = rpool.tile([_P, _KT], f32, tag="pref")
                    nc.vector.tensor_tensor(
                        pre_f[:], sf[:],
                        bfull_t[:, fi * _KT:(fi + 1) * _KT], add)
                    v = silu_split(pre_f, _KT, "v")

            # --- last row layer (identity activation, f32 out) ---
            pre = row_layer(_NRW - 1, v)
            nc.sync.dma_start(out.ap(), pre[:])

    nc.compile()
    return nc


def _prep_inputs(x, W, b):
    import ml_dtypes
    bf16 = ml_dtypes.bfloat16
    W = np.ascontiguousarray(W, dtype=np.float32)
    b = np.ascontiguousarray(b, dtype=np.float32)
    x = np.ascontiguousarray(x, dtype=np.float32)

    # row weights: [i6][cc][p, (kt, c, m)], elem = W[i][cc*256+c*128+m, p*16+kt]
    wrow = np.empty((_M, _NRW, _P, 2 * _KT * _P), dtype=bf16)
    for n, i in enumerate(_ROW_IDS):
        Wi = W[i].reshape(_M, 2, _P, _P, _KT)       # [cc, c, m, p, t]
        Wi = Wi.transpose(0, 3, 4, 1, 2)            # [cc, p, t, c, m]
        wrow[:, n] = Wi.reshape(_M, _P, 2 * _KT * _P).astype(bf16)

    # col weights: [i5][cc][p, (c, mt, m)], elem = W[i][m*16+mt, cc*256+c*128+p]
    wcol = np.empty((_M, _NE, _P, 2 * _KT * _P), dtype=bf16)
    for n, i in enumerate(_COL_IDS):
        Wi = W[i].reshape(_P, _KT, _M, 2, _P)       # [m, mt, cc, c, p]
        Wi = Wi.transpose(2, 4, 3, 1, 0)            # [cc, p, c, mt, m]
        wcol[:, n] = Wi.reshape(_M, _P, 2 * _KT * _P).astype(bf16)

    # full weights (replicated): [i4][p, (kt, mt, m)], elem = W[i][m*16+mt, p*16+kt]
    wfull = np.zeros((max(_NF, 1), _P, _KT * _KT * _P), dtype=bf16)
    for n, i in enumerate(_FULL_IDS):
        Wi = W[i].reshape(_P, _KT, _P, _KT)         # [m, mt, p, kt]
        Wi = Wi.transpose(2, 3, 1, 0)               # [p, kt, mt, m]
        wfull[n] = Wi.reshape(_P, _KT * _KT * _P).astype(bf16)

    # row biases: [cc][p, (i6, c)] f32 = b[i][cc*256 + c*128 + p]
    brow = np.empty((_M, _P, _NRW * 2), dtype=np.float32)
    for n, i in enumerate(_ROW_IDS):
        bi = b[i].reshape(_M, 2, _P)                # [cc, c, p]
        brow[:, :, 2 * n:2 * n + 2] = bi.transpose(0, 2, 1)

    # col/full biases (replicated): [p, (idx, t)] = b[i][p*16+t], f32
    bcol = np.empty((_P, _NE * _KT), dtype=np.float32)
    for n, i in enumerate(_COL_IDS):
        bcol[:, n * _KT:(n + 1) * _KT] = b[i].reshape(_P, _KT)
    bfull = np.zeros((_P, max(_NF, 1) * _KT), dtype=np.float32)
    for n, i in enumerate(_FULL_IDS):
        bfull[:, n * _KT:(n + 1) * _KT] = b[i].reshape(_P, _KT)

    # x hi/lo split: [p, (h, t)] with value x[p*16+t]
    xm = x.reshape(_P, _KT)
    xhi = xm.astype(bf16)
    xlo = (xm - xhi.astype(np.float32)).astype(bf16)
    x0 = np.concatenate([xhi, xlo], axis=1)

    in_maps = []
    for c in range(_M):
        in_maps.append({
            "wrow": np.ascontiguousarray(wrow[c]),
            "wcol": np.ascontiguousarray(wcol[c]),
            "wfull": wfull,
            "brow": np.ascontiguousarray(brow[c]),
            "bcol": bcol,
            "bfull": bfull,
            "x0": x0,
        })
    return in_maps


def kernel(x, W, b, _trace=False):
    from concourse.bass_utils import run_bass_kernel_spmd

    key = "nc"
    if key not in _CACHE:
        _CACHE[key] = _build()
    nc = _CACHE[key]

    in_maps = _prep_inputs(x, W, b)
    res = run_bass_kernel_spmd(
        nc, in_maps, core_ids=list(range(_M)), trace=_trace)
    _CACHE["last_results"] = res
    # out[p, c] holds global neuron core*256 + c*128 + p
    return np.concatenate(
        [res.results[c]["out"].T.reshape(-1) for c in range(_M)])


# revision 34
# speedup vs baseline: 1.2659x; 1.0008x over previous
"""Trainium2 Bass kernel for a 16-layer fully-connected chain (matvec per layer).

Computation (reference):
    v = x                       # [2048]
    for i in 0..13:  v = silu(W[i] @ v + b[i])
    out = W[14] @ v + b[14]

Strategy (8 NeuronCores):
  - Weights stream as bf16 (half the HBM traffic of f32). Activations are
    carried as a bf16 hi/lo pair (v = hi + lo, ~fp24 effective) riding the
    same stationary weight tile as two moving columns. Chain rel-err
    ~1.2e-2 (bf16-weight-limited) vs the 2e-2 gate.
  - Cross-core exchanges are the dominant serial cost (a collective has a
    large flat latency), so the schedule trades extra -- mostly hidden --
    weight DMA for fewer collectives. 15 layers run as 5 exchange segments
    with a fully-replicated layer opening segments 2-5:

    L0 r, L1 c, AG | L2 r, L3 c, AG, L4 F | L5 r, L6 c, AG, L7 F |
    L8 r, L9 c, AG, L10 F | L11 r, L12 c, AG, L13 F | L14 r
      * row layer: core c computes its 256 output neurons from the full
        activation vector (1/8 of W, no exchange).
      * col layer: core c multiplies its 256 local activations by the
        matching 256-column slice of W (1/8), producing a partial sum over
        all 2048 outputs; partials are AllGathered (8 x 8KB f32) and each
        core reduces them locally (one DVE tensor_reduce over the 8 rank
        slots plus a pre-filled bias slot), then silu.
      * full layer: every core redundantly computes the whole layer from
        the full weight matrix (4 MB bf16 per core instead of 0.5 MB).
        The extra DMA hides under the exchange chain; the exchange it
        replaces does not. 5 collectives total instead of 7.
  - All matmuls are weight-stationary (lhsT = [128k x 128m] tile, rhs =
    hi/lo pair [128, 2]); activations stay on 128 partitions end-to-end.
  - PSUM hi/lo combines are single DVE tensor_reduce ops over a [p, n, 2]
    view; biases are f32 tensors added on DVE (col-layer bias rides the
    exchange reduce as landing slot 9).

TimelineSim cost-model exec: ~154 us (prior 7-AG version: ~195 us;
original baseline row-sharded f32 kernel with 14 AllGathers: ~466 us).

Neuron-index conventions (baked into the host-side permutations):
  full vector:  sigma(p, t) = p*16 + t     (p = SBUF partition, t = column)
  local 256:    lam(c, m)   = c*128 + m    (c = psum column, m = partition)
"""

import numpy as np

_L = 15          # weight matrices
_N = 2048        # neurons per layer
_M = 8           # cores
_SH = _N // _M   # 256 local slice
_P = 128
_KT = _N // _P   # 16 k-chunks per full vector
_NE = 5          # exchanges (AllGathers)
_FULL_AFTER = [1, 2, 3, 4]   # segments followed by a replicated full layer

def _schedule(ne, full_after):
    rows, cols, fulls = [], [], []
    li = 0
    for seg in range(ne):
        rows.append(li); li += 1
        cols.append(li); li += 1
        if seg in full_after:
            fulls.append(li); li += 1
    rows.append(li); li += 1
    assert li == _L, li
    return rows, cols, fulls

_ROW_IDS, _COL_IDS, _FULL_IDS = _schedule(_NE, _FULL_AFTER)
_NF = len(_FULL_IDS)
_NRW = len(_ROW_IDS)

_CACHE = {}


def _build():
    import concourse.bacc as bacc
    import concourse.mybir as mybir
    import concourse.tile as tile

    f32 = mybir.dt.float32
    bf16 = mybir.dt.bfloat16
    silu = mybir.ActivationFunctionType.Silu
    add = mybir.AluOpType.add
    sub = mybir.AluOpType.subtract

    nc = bacc.Bacc("TRN2", target_bir_lowering=False, debug=False,
                   num_devices=_M)

    # row layers [6][p][(kt, c, m)]: elem = W[i][core*256+c*128+m, p*16+kt]
    wrow = nc.dram_tensor("wrow", [_NRW, _P, 2 * _KT * _P], bf16,
                          kind="ExternalInput")
    # col layers [5][p][(c, mt, m)]: elem = W[i][m*16+mt, core*256+c*128+p]
    wcol = nc.dram_tensor("wcol", [_NE, _P, 2 * _KT * _P], bf16,
                          kind="ExternalInput")
    # full layers [4][p][(kt, mt, m)]: elem = W[i][m*16+mt, p*16+kt]
    wfull = nc.dram_tensor("wfull", [max(_NF, 1), _P, _KT * _KT * _P],
                           bf16, kind="ExternalInput")
    # row biases [p, (i6, c)] f32: value b[i][core*256 + c*128 + p]
    brow = nc.dram_tensor("brow", [_P, _NRW * 2], f32, kind="ExternalInput")
    # col biases [p, (i5, t)] f32: value b[i][sigma(p,t)]
    bcol = nc.dram_tensor("bcol", [_P, _NE * _KT], f32, kind="ExternalInput")
    # full biases [p, (i4, t)] f32: value b[i][sigma(p,t)]
    bfull = nc.dram_tensor("bfull", [_P, max(_NF, 1) * _KT], f32,
                           kind="ExternalInput")
    # x in full-vector layout, hi/lo split: [p, (h, t)] bf16
    x0 = nc.dram_tensor("x0", [_P, 2 * _KT], bf16, kind="ExternalInput")
    # last row layer output [p, c] f32 (global j = core*256 + c*128 + p)
    out = nc.dram_tensor("out", [_P, 2], f32, kind="ExternalOutput")

    with tile.TileContext(nc) as tc:
        with (
            tc.tile_pool(name="w", bufs=8) as wpool,
            tc.tile_pool(name="wf", bufs=2) as wfhpool,
            tc.tile_pool(name="v", bufs=3) as vpool,
            tc.tile_pool(name="s", bufs=4) as spool,
            tc.tile_pool(name="r", bufs=4) as rpool,
            tc.tile_pool(name="consts", bufs=1) as cpool,
            tc.tile_pool(name="psr", bufs=2, space="PSUM") as psrpool,
            tc.tile_pool(name="psc", bufs=2, space="PSUM") as pscpool,
            tc.tile_pool(name="dram", bufs=2, space="DRAM") as dpool,
        ):
            w_first = wpool.tile([_P, 2 * _KT * _P], bf16, tag="w")
            nc.sync.dma_start(w_first[:], wrow.ap()[0])
            wc_first = wpool.tile([_P, 2 * _KT * _P], bf16, tag="w")
            nc.sync.dma_start(wc_first[:], wcol.ap()[0])

            brow_t = cpool.tile([_P, _NRW * 2], f32)
            nc.sync.dma_start(brow_t[:], brow.ap())
            bcol_t = cpool.tile([_P, _NE * _KT], f32)
            nc.sync.dma_start(bcol_t[:], bcol.ap())
            bfull_t = cpool.tile([_P, _NF * _KT], f32)
            nc.sync.dma_start(bfull_t[:], bfull.ap())

            # persistent landing area: per exchange, 8 rank slots + bias
            # slot 9 (pre-filled once; the reduce then covers the bias)
            _SL = (_M + 1) * _KT  # 144 floats per exchange
            landall = cpool.tile([_P, _NE * _SL], f32)
            nc.vector.tensor_copy(
                landall[:].rearrange("p (j s) -> p j s", j=_NE)[:, :, _M * _KT:],
                bcol_t[:].rearrange("p (j t) -> p j t", j=_NE))

            v = vpool.tile([_P, 2 * _KT], bf16, tag="v")
            nc.sync.dma_start(v[:], x0.ap())

            def hi_lo_reduce(ps, n, tag):
                """PSUM [p, (n, h)] -> SBUF f32 [p, n] (hi+lo summed)."""
                sm = rpool.tile([_P, n], f32, tag=tag)
                nc.vector.tensor_reduce(
                    sm[:],
                    ps[:].rearrange("p (n h) -> p n h", h=2),
                    axis=mybir.AxisListType.X,
                    op=add,
                )
                return sm

            def silu_split(pre, n, tag):
                """f32 [p, n] -> bf16 [p, (h, n)] hi/lo pair of silu."""
                s32 = spool.tile([_P, n], f32, tag=tag + "32")
                nc.scalar.activation(s32[:], pre[:], silu)
                s = spool.tile([_P, 2 * n], bf16, tag=tag)
                nc.vector.tensor_copy(s[:, 0:n], s32[:])
                nc.vector.tensor_tensor(s[:, n:2 * n], s32[:], s[:, 0:n], sub)
                return s

            def row_layer(i6, v, w=None):
                """v [p,(h,t)] bf16 -> pre [p, 2] f32 (bias added)."""
                if w is None:
                    w = wpool.tile([_P, 2 * _KT * _P], bf16, tag="w")
                    nc.sync.dma_start(w[:], wrow.ap()[i6])
                ps = psrpool.tile([_P, 4], f32, tag="psr")
                for c in range(2):
                    for kt in range(_KT):
                        f = (kt * 2 + c) * _P
                        nc.tensor.matmul(
                            ps[:, 2 * c:2 * c + 2],
                            lhsT=w[:, f:f + _P],
                            rhs=v[:, kt:kt + 17:16],
                            start=(kt == 0),
                            stop=(kt == _KT - 1),
                        )
                sm = hi_lo_reduce(ps, 2, "rsum")
                pre = rpool.tile([_P, 2], f32, tag="prer")
                nc.vector.tensor_tensor(
                    pre[:], sm[:], brow_t[:, i6 * 2:i6 * 2 + 2], add)
                return pre

            for seg in range(_NE):
                # --- row layer ---
                pre = row_layer(seg, v, w=w_first if seg == 0 else None)
                s = silu_split(pre, 2, "s")

                # --- col layer: partial over all 2048 outputs ---
                if seg == 0:
                    w = wc_first
                else:
                    w = wpool.tile([_P, 2 * _KT * _P], bf16, tag="w")
                    nc.sync.dma_start(w[:], wcol.ap()[seg])
                pc = pscpool.tile([_P, 2 * _KT], f32, tag="psc")
                for mt in range(_KT):
                    for c in range(2):
                        f = (c * _KT + mt) * _P
                        nc.tensor.matmul(
                            pc[:, 2 * mt:2 * mt + 2],
                            lhsT=w[:, f:f + _P],
                            rhs=s[:, c:c + 3:2],
                            start=(c == 0),
                            stop=(c == 1),
                        )
                sp = hi_lo_reduce(pc, _KT, "sp")

                # --- exchange: AllGather partials, reduce + bias + silu ---
                cc_in = dpool.tile([_P, _KT], f32, tag="ccin")
                nc.sync.dma_start(cc_in[:], sp[:])
                cc_out = dpool.tile([_M, _N], f32, tag="ccout")
                nc.gpsimd.collective_compute(
                    "AllGather",
                    mybir.AluOpType.bypass,
                    replica_groups=[list(range(_M))],
                    ins=[cc_in.opt()],
                    outs=[cc_out.opt()],
                )
                land = landall[:, seg * _SL:seg * _SL + _M * _KT]
                nc.sync.dma_start(
                    land.rearrange("p (r t) -> p r t", r=_M),
                    cc_out[:, :].rearrange("r (p t) -> p r t", p=_P))
                pre_v = rpool.tile([_P, _KT], f32, tag="prev")
                nc.vector.tensor_reduce(
                    pre_v[:],
                    landall[:, seg * _SL:(seg + 1) * _SL]
                    .rearrange("p (r t) -> p t r", r=_M + 1),
                    axis=mybir.AxisListType.X,
                    op=add,
                )
                v = silu_split(pre_v, _KT, "v")

                # --- full layer (replicated on every core) ---
                if seg in _FULL_AFTER:
                    fi = _FULL_AFTER.index(seg)
                    wf = wfhpool.tile([_P, _KT * _KT * _P], bf16, tag="wf")
                    # split the 4MB load into 4 chunks so a latency-critical
                    # exchange bounce DMA never queues behind more than ~6us
                    # of weight transfer on the SP DMA path
                    _ck = _KT * _KT * _P // 4
                    for _c in range(4):
                        nc.sync.dma_start(
                            wf[:, _c * _ck:(_c + 1) * _ck],
                            wfull.ap()[fi][:, _c * _ck:(_c + 1) * _ck])
                    pf = pscpool.tile([_P, 2 * _KT], f32, tag="psf")
                    for mt in range(_KT):
                        for kt in range(_KT):
                            f = (kt * _KT + mt) * _P
                            nc.tensor.matmul(
                                pf[:, 2 * mt:2 * mt + 2],
                                lhsT=wf[:, f:f + _P],
                                rhs=v[:, kt:kt + 17:16],
                                start=(kt == 0),
                                stop=(kt == _KT - 1),
                            )
                    sf = hi_lo_reduce(pf, _KT, "sf")
                    pre_f = rpool.tile([_P, _KT], f32, tag="pref")
                    nc.vector.tensor_tensor(
                        pre_f[:], sf[:],
                        bfull_t[:, fi * _KT:(fi + 1) * _KT], add)
                    v = silu_split(pre_f, _KT, "v")

            # --- last row layer (identity activation, f32 out) ---
            pre = row_layer(_NRW - 1, v)
            nc.sync.dma_start(out.ap(), pre[:])

    nc.compile()
    return nc


def _prep_inputs(x, W, b):
    import ml_dtypes
    bf16 = ml_dtypes.bfloat16
    W = np.ascontiguousarray(W, dtype=np.float32)
    b = np.ascontiguousarray(b, dtype=np.float32)
    x = np.ascontiguousarray(x, dtype=np.float32)

    # row weights: [i6][cc][p, (kt, c, m)], elem = W[i][cc*256+c*128+m, p*16+kt]
    wrow = np.empty((_M, _NRW, _P, 2 * _KT * _P), dtype=bf16)
    for n, i in enumerate(_ROW_IDS):
        Wi = W[i].reshape(_M, 2, _P, _P, _KT)       # [cc, c, m, p, t]
        Wi = Wi.transpose(0, 3, 4, 1, 2)            # [cc, p, t, c, m]
        wrow[:, n] = Wi.reshape(_M, _P, 2 * _KT * _P).astype(bf16)

    # col weights: [i5][cc][p, (c, mt, m)], elem = W[i][m*16+mt, cc*256+c*128+p]
    wcol = np.empty((_M, _NE, _P, 2 * _KT * _P), dtype=bf16)
    for n, i in enumerate(_COL_IDS):
        Wi = W[i].reshape(_P, _KT, _M, 2, _P)       # [m, mt, cc, c, p]
        Wi = Wi.transpose(2, 4, 3, 1, 0)            # [cc, p, c, mt, m]
        wcol[:, n] = Wi.reshape(_M, _P, 2 * _KT * _P).astype(bf16)

    # full weights (replicated): [i4][p, (kt, mt, m)], elem = W[i][m*16+mt, p*16+kt]
    wfull = np.zeros((max(_NF, 1), _P, _KT * _KT * _P), dtype=bf16)
    for n, i in enumerate(_FULL_IDS):
        Wi = W[i].reshape(_P, _KT, _P, _KT)         # [m, mt, p, kt]
        Wi = Wi.transpose(2, 3, 1, 0)               # [p, kt, mt, m]
        wfull[n] = Wi.reshape(_P, _KT * _KT * _P).astype(bf16)

    # row biases: [cc][p, (i6, c)] f32 = b[i][cc*256 + c*128 + p]
    brow = np.empty((_M, _P, _NRW * 2), dtype=np.float32)
    for n, i in enumerate(_ROW_IDS):
        bi = b[i].reshape(_M, 2, _P)                # [cc, c, p]
        brow[:, :, 2 * n:2 * n + 2] = bi.transpose(0, 2, 1)

    # col/full biases (replicated): [p, (idx, t)] = b[i][p*16+t], f32
    bcol = np.empty((_P, _NE * _KT), dtype=np.float32)
    for n, i in enumerate(_COL_IDS):
        bcol[:, n * _KT:(n + 1) * _KT] = b[i].reshape(_P, _KT)
    bfull = np.zeros((_P, max(_NF, 1) * _KT), dtype=np.float32)
    for n, i in enumerate(_FULL_IDS):
        bfull[:, n * _KT:(n + 1) * _KT] = b[i].reshape(_P, _KT)

    # x hi/lo split: [p, (h, t)] with value x[p*16+t]
    xm = x.reshape(_P, _KT)
    xhi = xm.astype(bf16)
    xlo = (xm - xhi.astype(np.float32)).astype(bf16)
    x0 = np.concatenate([xhi, xlo], axis=1)

    in_maps = []
    for c in range(_M):
        in_maps.append({
            "wrow": np.ascontiguousarray(wrow[c]),
            "wcol": np.ascontiguousarray(wcol[c]),
            "wfull": wfull,
            "brow": np.ascontiguousarray(brow[c]),
            "bcol": bcol,
            "bfull": bfull,
            "x0": x0,
        })
    return in_maps


def kernel(x, W, b, _trace=False):
    from concourse.bass_utils import run_bass_kernel_spmd

    key = "nc"
    if key not in _CACHE:
        _CACHE[key] = _build()
    nc = _CACHE[key]

    in_maps = _prep_inputs(x, W, b)
    res = run_bass_kernel_spmd(
        nc, in_maps, core_ids=list(range(_M)), trace=_trace)
    _CACHE["last_results"] = res
    # out[p, c] holds global neuron core*256 + c*128 + p
    return np.concatenate(
        [res.results[c]["out"].T.reshape(-1) for c in range(_M)])


# revision 35
# speedup vs baseline: 1.2963x; 1.0240x over previous
"""Trainium2 Bass kernel for a 16-layer fully-connected chain (matvec per layer).

Computation (reference):
    v = x                       # [2048]
    for i in 0..13:  v = silu(W[i] @ v + b[i])
    out = W[14] @ v + b[14]

Strategy (8 NeuronCores):
  - Weights stream as bf16 (half the HBM traffic of f32). Activations are
    carried as a bf16 hi/lo pair (v = hi + lo, ~fp24 effective) riding the
    same stationary weight tile as two moving columns. Chain rel-err
    ~1.2e-2 (bf16-weight-limited) vs the 2e-2 gate.
  - Cross-core exchanges are the dominant serial cost (a collective has a
    large flat latency), so the schedule trades extra -- mostly hidden --
    weight DMA for fewer collectives. 15 layers run as 5 exchange segments
    with a fully-replicated layer opening segments 2-5:

    L0 r, L1 c, AG | L2 r, L3 c, AG, L4 F | L5 r, L6 c, AG, L7 F |
    L8 r, L9 c, AG, L10 F | L11 r, L12 c, AG, L13 F | L14 r
      * row layer: core c computes its 256 output neurons from the full
        activation vector (1/8 of W, no exchange).
      * col layer: core c multiplies its 256 local activations by the
        matching 256-column slice of W (1/8), producing a partial sum over
        all 2048 outputs; partials are AllGathered (8 x 8KB f32) and each
        core reduces them locally (one DVE tensor_reduce over the 8 rank
        slots plus a pre-filled bias slot), then silu.
      * full layer: every core redundantly computes the whole layer from
        the full weight matrix (4 MB bf16 per core instead of 0.5 MB).
        The extra DMA hides under the exchange chain; the exchange it
        replaces does not. 5 collectives total instead of 7.
  - All matmuls are weight-stationary (lhsT = [128k x 128m] tile, rhs =
    hi/lo pair [128, 2]); activations stay on 128 partitions end-to-end.
  - PSUM hi/lo combines are single DVE tensor_reduce ops over a [p, n, 2]
    view; biases are f32 tensors added on DVE (col-layer bias rides the
    exchange reduce as landing slot 9).

TimelineSim cost-model exec: ~154 us (prior 7-AG version: ~195 us;
original baseline row-sharded f32 kernel with 14 AllGathers: ~466 us).

Neuron-index conventions (baked into the host-side permutations):
  full vector:  sigma(p, t) = p*16 + t     (p = SBUF partition, t = column)
  local 256:    lam(c, m)   = c*128 + m    (c = psum column, m = partition)
"""

import numpy as np

_L = 15          # weight matrices
_N = 2048        # neurons per layer
_M = 8           # cores
_SH = _N // _M   # 256 local slice
_P = 128
_KT = _N // _P   # 16 k-chunks per full vector
_NE = 5          # exchanges (AllGathers)
_FULL_AFTER = [1, 2, 3, 4]   # segments followed by a replicated full layer

def _schedule(ne, full_after):
    rows, cols, fulls = [], [], []
    li = 0
    for seg in range(ne):
        rows.append(li); li += 1
        cols.append(li); li += 1
        if seg in full_after:
            fulls.append(li); li += 1
    rows.append(li); li += 1
    assert li == _L, li
    return rows, cols, fulls

_ROW_IDS, _COL_IDS, _FULL_IDS = _schedule(_NE, _FULL_AFTER)
_NF = len(_FULL_IDS)
_NRW = len(_ROW_IDS)

_CACHE = {}


def _build():
    import concourse.bacc as bacc
    import concourse.mybir as mybir
    import concourse.tile as tile

    f32 = mybir.dt.float32
    bf16 = mybir.dt.bfloat16
    silu = mybir.ActivationFunctionType.Silu
    add = mybir.AluOpType.add
    sub = mybir.AluOpType.subtract

    nc = bacc.Bacc("TRN2", target_bir_lowering=False, debug=False,
                   num_devices=_M)

    # row layers [6][p][(kt, c, m)]: elem = W[i][core*256+c*128+m, p*16+kt]
    wrow = nc.dram_tensor("wrow", [_NRW, _P, 2 * _KT * _P], bf16,
                          kind="ExternalInput")
    # col layers [5][p][(c, mt, m)]: elem = W[i][m*16+mt, core*256+c*128+p]
    wcol = nc.dram_tensor("wcol", [_NE, _P, 2 * _KT * _P], bf16,
                          kind="ExternalInput")
    # full layers [4][p][(kt, mt, m)]: elem = W[i][m*16+mt, p*16+kt]
    wfull = nc.dram_tensor("wfull", [max(_NF, 1), _P, _KT * _KT * _P],
                           bf16, kind="ExternalInput")
    # row biases [p, (i6, c)] f32: value b[i][core*256 + c*128 + p]
    brow = nc.dram_tensor("brow", [_P, _NRW * 2], f32, kind="ExternalInput")
    # col biases [p, (i5, t)] f32: value b[i][sigma(p,t)]
    bcol = nc.dram_tensor("bcol", [_P, _NE * _KT], f32, kind="ExternalInput")
    # full biases [p, (i4, t)] f32: value b[i][sigma(p,t)]
    bfull = nc.dram_tensor("bfull", [_P, max(_NF, 1) * _KT], f32,
                           kind="ExternalInput")
    # x in full-vector layout, hi/lo split: [p, (h, t)] bf16
    x0 = nc.dram_tensor("x0", [_P, 2 * _KT], bf16, kind="ExternalInput")
    # last row layer output [p, c] f32 (global j = core*256 + c*128 + p)
    out = nc.dram_tensor("out", [_P, 2], f32, kind="ExternalOutput")

    with tile.TileContext(nc) as tc:
        with (
            tc.tile_pool(name="w", bufs=8) as wpool,
            tc.tile_pool(name="wf", bufs=2) as wfhpool,
            tc.tile_pool(name="v", bufs=3) as vpool,
            tc.tile_pool(name="s", bufs=4) as spool,
            tc.tile_pool(name="r", bufs=4) as rpool,
            tc.tile_pool(name="consts", bufs=1) as cpool,
            tc.tile_pool(name="psr", bufs=2, space="PSUM") as psrpool,
            tc.tile_pool(name="psc", bufs=2, space="PSUM") as pscpool,
            tc.tile_pool(name="dram", bufs=2, space="DRAM") as dpool,
        ):
            w_first = wpool.tile([_P, 2 * _KT * _P], bf16, tag="w")
            nc.sync.dma_start(w_first[:], wrow.ap()[0])
            wc_first = wpool.tile([_P, 2 * _KT * _P], bf16, tag="w")
            nc.sync.dma_start(wc_first[:], wcol.ap()[0])

            brow_t = cpool.tile([_P, _NRW * 2], f32)
            nc.sync.dma_start(brow_t[:], brow.ap())
            bcol_t = cpool.tile([_P, _NE * _KT], f32)
            nc.sync.dma_start(bcol_t[:], bcol.ap())
            bfull_t = cpool.tile([_P, _NF * _KT], f32)
            nc.sync.dma_start(bfull_t[:], bfull.ap())

            # persistent landing area: per exchange, 8 rank slots + bias
            # slot 9 (pre-filled once; the reduce then covers the bias)
            _SL = (_M + 1) * _KT  # 144 floats per exchange
            landall = cpool.tile([_P, _NE * _SL], bf16)
            nc.vector.tensor_copy(
                landall[:].rearrange("p (j s) -> p j s", j=_NE)[:, :, _M * _KT:],
                bcol_t[:].rearrange("p (j t) -> p j t", j=_NE))

            v = vpool.tile([_P, 2 * _KT], bf16, tag="v")
            nc.sync.dma_start(v[:], x0.ap())

            def hi_lo_reduce(ps, n, tag):
                """PSUM [p, (n, h)] -> SBUF f32 [p, n] (hi+lo summed)."""
                sm = rpool.tile([_P, n], f32, tag=tag)
                nc.vector.tensor_reduce(
                    sm[:],
                    ps[:].rearrange("p (n h) -> p n h", h=2),
                    axis=mybir.AxisListType.X,
                    op=add,
                )
                return sm

            def silu_split(pre, n, tag):
                """f32 [p, n] -> bf16 [p, (h, n)] hi/lo pair of silu."""
                s32 = spool.tile([_P, n], f32, tag=tag + "32")
                nc.scalar.activation(s32[:], pre[:], silu)
                s = spool.tile([_P, 2 * n], bf16, tag=tag)
                nc.vector.tensor_copy(s[:, 0:n], s32[:])
                nc.vector.tensor_tensor(s[:, n:2 * n], s32[:], s[:, 0:n], sub)
                return s

            def row_layer(i6, v, w=None):
                """v [p,(h,t)] bf16 -> pre [p, 2] f32 (bias added)."""
                if w is None:
                    w = wpool.tile([_P, 2 * _KT * _P], bf16, tag="w")
                    nc.sync.dma_start(w[:], wrow.ap()[i6])
                ps = psrpool.tile([_P, 4], f32, tag="psr")
                for c in range(2):
                    for kt in range(_KT):
                        f = (kt * 2 + c) * _P
                        nc.tensor.matmul(
                            ps[:, 2 * c:2 * c + 2],
                            lhsT=w[:, f:f + _P],
                            rhs=v[:, kt:kt + 17:16],
                            start=(kt == 0),
                            stop=(kt == _KT - 1),
                        )
                sm = hi_lo_reduce(ps, 2, "rsum")
                pre = rpool.tile([_P, 2], f32, tag="prer")
                nc.vector.tensor_tensor(
                    pre[:], sm[:], brow_t[:, i6 * 2:i6 * 2 + 2], add)
                return pre

            for seg in range(_NE):
                # --- row layer ---
                pre = row_layer(seg, v, w=w_first if seg == 0 else None)
                s = silu_split(pre, 2, "s")

                # --- col layer: partial over all 2048 outputs ---
                if seg == 0:
                    w = wc_first
                else:
                    w = wpool.tile([_P, 2 * _KT * _P], bf16, tag="w")
                    nc.sync.dma_start(w[:], wcol.ap()[seg])
                pc = pscpool.tile([_P, 2 * _KT], f32, tag="psc")
                for mt in range(_KT):
                    for c in range(2):
                        f = (c * _KT + mt) * _P
                        nc.tensor.matmul(
                            pc[:, 2 * mt:2 * mt + 2],
                            lhsT=w[:, f:f + _P],
                            rhs=s[:, c:c + 3:2],
                            start=(c == 0),
                            stop=(c == 1),
                        )
                sp32 = hi_lo_reduce(pc, _KT, "sp32")
                sp = spool.tile([_P, _KT], bf16, tag="spb")
                nc.vector.tensor_copy(sp[:], sp32[:])

                # --- exchange: AllGather partials, reduce + bias + silu ---
                cc_in = dpool.tile([_P, _KT], bf16, tag="ccin")
                nc.sync.dma_start(cc_in[:], sp[:])
                cc_out = dpool.tile([_M, _N], bf16, tag="ccout")
                nc.gpsimd.collective_compute(
                    "AllGather",
                    mybir.AluOpType.bypass,
                    replica_groups=[list(range(_M))],
                    ins=[cc_in.opt()],
                    outs=[cc_out.opt()],
                )
                land = landall[:, seg * _SL:seg * _SL + _M * _KT]
                nc.sync.dma_start(
                    land.rearrange("p (r t) -> p r t", r=_M),
                    cc_out[:, :].rearrange("r (p t) -> p r t", p=_P))
                pre_v = rpool.tile([_P, _KT], f32, tag="prev")
                nc.vector.tensor_reduce(
                    pre_v[:],
                    landall[:, seg * _SL:(seg + 1) * _SL]
                    .rearrange("p (r t) -> p t r", r=_M + 1),
                    axis=mybir.AxisListType.X,
                    op=add,
                )
                v = silu_split(pre_v, _KT, "v")

                # --- full layer (replicated on every core) ---
                if seg in _FULL_AFTER:
                    fi = _FULL_AFTER.index(seg)
                    wf = wfhpool.tile([_P, _KT * _KT * _P], bf16, tag="wf")
                    # split the 4MB load into 4 chunks so a latency-critical
                    # exchange bounce DMA never queues behind more than ~6us
                    # of weight transfer on the SP DMA path
                    _ck = _KT * _KT * _P // 4
                    for _c in range(4):
                        nc.sync.dma_start(
                            wf[:, _c * _ck:(_c + 1) * _ck],
                            wfull.ap()[fi][:, _c * _ck:(_c + 1) * _ck])
                    pf = pscpool.tile([_P, 2 * _KT], f32, tag="psf")
                    for mt in range(_KT):
                        for kt in range(_KT):
                            f = (kt * _KT + mt) * _P
                            nc.tensor.matmul(
                                pf[:, 2 * mt:2 * mt + 2],
                                lhsT=wf[:, f:f + _P],
                                rhs=v[:, kt:kt + 17:16],
                                start=(kt == 0),
                                stop=(kt == _KT - 1),
                            )
                    sf = hi_lo_reduce(pf, _KT, "sf")
                    pre_f = rpool.tile([_P, _KT], f32, tag="pref")
                    nc.vector.tensor_tensor(
                        pre_f[:], sf[:],
                        bfull_t[:, fi * _KT:(fi + 1) * _KT], add)
                    v = silu_split(pre_f, _KT, "v")

            # --- last row layer (identity activation, f32 out) ---
            pre = row_layer(_NRW - 1, v)
            nc.sync.dma_start(out.ap(), pre[:])

    nc.compile()
    return nc


def _prep_inputs(x, W, b):
    import ml_dtypes
    bf16 = ml_dtypes.bfloat16
    W = np.ascontiguousarray(W, dtype=np.float32)
    b = np.ascontiguousarray(b, dtype=np.float32)
    x = np.ascontiguousarray(x, dtype=np.float32)

    # row weights: [i6][cc][p, (kt, c, m)], elem = W[i][cc*256+c*128+m, p*16+kt]
    wrow = np.empty((_M, _NRW, _P, 2 * _KT * _P), dtype=bf16)
    for n, i in enumerate(_ROW_IDS):
        Wi = W[i].reshape(_M, 2, _P, _P, _KT)       # [cc, c, m, p, t]
        Wi = Wi.transpose(0, 3, 4, 1, 2)            # [cc, p, t, c, m]
        wrow[:, n] = Wi.reshape(_M, _P, 2 * _KT * _P).astype(bf16)

    # col weights: [i5][cc][p, (c, mt, m)], elem = W[i][m*16+mt, cc*256+c*128+p]
    wcol = np.empty((_M, _NE, _P, 2 * _KT * _P), dtype=bf16)
    for n, i in enumerate(_COL_IDS):
        Wi = W[i].reshape(_P, _KT, _M, 2, _P)       # [m, mt, cc, c, p]
        Wi = Wi.transpose(2, 4, 3, 1, 0)            # [cc, p, c, mt, m]
        wcol[:, n] = Wi.reshape(_M, _P, 2 * _KT * _P).astype(bf16)

    # full weights (replicated): [i4][p, (kt, mt, m)], elem = W[i][m*16+mt, p*16+kt]
    wfull = np.zeros((max(_NF, 1), _P, _KT * _KT * _P), dtype=bf16)
    for n, i in enumerate(_FULL_IDS):
        Wi = W[i].reshape(_P, _KT, _P, _KT)         # [m, mt, p, kt]
        Wi = Wi.transpose(2, 3, 1, 0)               # [p, kt, mt, m]
        wfull[n] = Wi.reshape(_P, _KT * _KT * _P).astype(bf16)

    # row biases: [cc][p, (i6, c)] f32 = b[i][cc*256 + c*128 + p]
    brow = np.empty((_M, _P, _NRW * 2), dtype=np.float32)
    for n, i in enumerate(_ROW_IDS):
        bi = b[i].reshape(_M, 2, _P)                # [cc, c, p]
        brow[:, :, 2 * n:2 * n + 2] = bi.transpose(0, 2, 1)

    # col/full biases (replicated): [p, (idx, t)] = b[i][p*16+t], f32
    bcol = np.empty((_P, _NE * _KT), dtype=np.float32)
    for n, i in enumerate(_COL_IDS):
        bcol[:, n * _KT:(n + 1) * _KT] = b[i].reshape(_P, _KT)
    bfull = np.zeros((_P, max(_NF, 1) * _KT), dtype=np.float32)
    for n, i in enumerate(_FULL_IDS):
        bfull[:, n * _KT:(n + 1) * _KT] = b[i].reshape(_P, _KT)

    # x hi/lo split: [p, (h, t)] with value x[p*16+t]
    xm = x.reshape(_P, _KT)
    xhi = xm.astype(bf16)
    xlo = (xm - xhi.astype(np.float32)).astype(bf16)
    x0 = np.concatenate([xhi, xlo], axis=1)

    in_maps = []
    for c in range(_M):
        in_maps.append({
            "wrow": np.ascontiguousarray(wrow[c]),
            "wcol": np.ascontiguousarray(wcol[c]),
            "wfull": wfull,
            "brow": np.ascontiguousarray(brow[c]),
            "bcol": bcol,
            "bfull": bfull,
            "x0": x0,
        })
    return in_maps


def kernel(x, W, b, _trace=False):
    from concourse.bass_utils import run_bass_kernel_spmd

    key = "nc"
    if key not in _CACHE:
        _CACHE[key] = _build()
    nc = _CACHE[key]

    in_maps = _prep_inputs(x, W, b)
    res = run_bass_kernel_spmd(
        nc, in_maps, core_ids=list(range(_M)), trace=_trace)
    _CACHE["last_results"] = res
    # out[p, c] holds global neuron core*256 + c*128 + p
    return np.concatenate(
        [res.results[c]["out"].T.reshape(-1) for c in range(_M)])


# revision 37
# speedup vs baseline: 1.3005x; 1.0033x over previous
"""Trainium2 Bass kernel for a 16-layer fully-connected chain (matvec per layer).

Computation (reference):
    v = x                       # [2048]
    for i in 0..13:  v = silu(W[i] @ v + b[i])
    out = W[14] @ v + b[14]

Strategy (8 NeuronCores):
  - Weights stream as bf16 (half the HBM traffic of f32). Activations are
    carried as a bf16 hi/lo pair (v = hi + lo, ~fp24 effective) riding the
    same stationary weight tile as two moving columns. Chain rel-err
    ~1.2e-2 (bf16-weight-limited) vs the 2e-2 gate.
  - Cross-core exchanges are the dominant serial cost (a collective has a
    large flat latency), so the schedule trades extra -- mostly hidden --
    weight DMA for fewer collectives. 15 layers run as 5 exchange segments
    with a fully-replicated layer opening segments 2-5:

    L0 r, L1 c, AG | L2 r, L3 c, AG, L4 F | L5 r, L6 c, AG, L7 F |
    L8 r, L9 c, AG, L10 F | L11 r, L12 c, AG, L13 F | L14 r
      * row layer: core c computes its 256 output neurons from the full
        activation vector (1/8 of W, no exchange).
      * col layer: core c multiplies its 256 local activations by the
        matching 256-column slice of W (1/8), producing a partial sum over
        all 2048 outputs; partials are AllGathered (8 x 4KB bf16 -- the
        hi/lo-split activations re-round to bf16 anyway, so exchanging
        rounded partials costs ~0.1% extra rel-err) and each core reduces
        them locally in f32 (one DVE tensor_reduce over the 8 rank slots
        plus a pre-filled bias slot), then silu.
      * full layer: every core redundantly computes the whole layer from
        the full weight matrix (4 MB bf16 per core instead of 0.5 MB).
        The extra DMA hides under the exchange chain; the exchange it
        replaces does not. 5 collectives total instead of 7.
  - All matmuls are weight-stationary (lhsT = [128k x 128m] tile, rhs =
    hi/lo pair [128, 2]); activations stay on 128 partitions end-to-end.
  - PSUM hi/lo combines are single DVE tensor_reduce ops over a [p, n, 2]
    view; biases are f32 tensors added on DVE (col-layer bias rides the
    exchange reduce as landing slot 9).

TimelineSim cost-model exec: ~150 us (prior 7-AG version: ~195 us;
original baseline row-sharded f32 kernel with 14 AllGathers: ~466 us).
The DMA engines are ~85% occupied end-to-end: the kernel is within ~15%
of the cost model's weight-streaming floor for this schedule.

Neuron-index conventions (baked into the host-side permutations):
  full vector:  sigma(p, t) = p*16 + t     (p = SBUF partition, t = column)
  local 256:    lam(c, m)   = c*128 + m    (c = psum column, m = partition)
"""

import numpy as np

_L = 15          # weight matrices
_N = 2048        # neurons per layer
_M = 8           # cores
_SH = _N // _M   # 256 local slice
_P = 128
_KT = _N // _P   # 16 k-chunks per full vector
_NE = 5          # exchanges (AllGathers)
_FULL_AFTER = [1, 2, 3, 4]   # segments followed by a replicated full layer

def _schedule(ne, full_after):
    rows, cols, fulls = [], [], []
    li = 0
    for seg in range(ne):
        rows.append(li); li += 1
        cols.append(li); li += 1
        if seg in full_after:
            fulls.append(li); li += 1
    rows.append(li); li += 1
    assert li == _L, li
    return rows, cols, fulls

_ROW_IDS, _COL_IDS, _FULL_IDS = _schedule(_NE, _FULL_AFTER)
_NF = len(_FULL_IDS)
_NRW = len(_ROW_IDS)

_CACHE = {}


def _build():
    import concourse.bacc as bacc
    import concourse.mybir as mybir
    import concourse.tile as tile

    f32 = mybir.dt.float32
    bf16 = mybir.dt.bfloat16
    silu = mybir.ActivationFunctionType.Silu
    add = mybir.AluOpType.add
    sub = mybir.AluOpType.subtract

    nc = bacc.Bacc("TRN2", target_bir_lowering=False, debug=False,
                   num_devices=_M)

    # row layers [6][p][(kt, c, m)]: elem = W[i][core*256+c*128+m, p*16+kt]
    wrow = nc.dram_tensor("wrow", [_NRW, _P, 2 * _KT * _P], bf16,
                          kind="ExternalInput")
    # col layers [5][p][(c, mt, m)]: elem = W[i][m*16+mt, core*256+c*128+p]
    wcol = nc.dram_tensor("wcol", [_NE, _P, 2 * _KT * _P], bf16,
                          kind="ExternalInput")
    # full layers [4][p][(kt, mt, m)]: elem = W[i][m*16+mt, p*16+kt]
    wfull = nc.dram_tensor("wfull", [max(_NF, 1), _P, _KT * _KT * _P],
                           bf16, kind="ExternalInput")
    # row biases [p, (i6, c)] f32: value b[i][core*256 + c*128 + p]
    brow = nc.dram_tensor("brow", [_P, _NRW * 2], f32, kind="ExternalInput")
    # col biases [p, (i5, t)] f32: value b[i][sigma(p,t)]
    bcol = nc.dram_tensor("bcol", [_P, _NE * _KT], f32, kind="ExternalInput")
    # full biases [p, (i4, t)] f32: value b[i][sigma(p,t)]
    bfull = nc.dram_tensor("bfull", [_P, max(_NF, 1) * _KT], f32,
                           kind="ExternalInput")
    # x in full-vector layout, hi/lo split: [p, (h, t)] bf16
    x0 = nc.dram_tensor("x0", [_P, 2 * _KT], bf16, kind="ExternalInput")
    # last row layer output [p, c] f32 (global j = core*256 + c*128 + p)
    out = nc.dram_tensor("out", [_P, 2], f32, kind="ExternalOutput")

    with tile.TileContext(nc) as tc:
        with (
            tc.tile_pool(name="w", bufs=8) as wpool,
            tc.tile_pool(name="wf", bufs=2) as wfhpool,
            tc.tile_pool(name="v", bufs=3) as vpool,
            tc.tile_pool(name="s", bufs=4) as spool,
            tc.tile_pool(name="r", bufs=4) as rpool,
            tc.tile_pool(name="consts", bufs=1) as cpool,
            tc.tile_pool(name="psr", bufs=2, space="PSUM") as psrpool,
            tc.tile_pool(name="psc", bufs=2, space="PSUM") as pscpool,
            tc.tile_pool(name="dram", bufs=2, space="DRAM") as dpool,
        ):
            w_first = wpool.tile([_P, 2 * _KT * _P], bf16, tag="w")
            nc.sync.dma_start(w_first[:], wrow.ap()[0])
            wc_first = wpool.tile([_P, 2 * _KT * _P], bf16, tag="w")
            nc.sync.dma_start(wc_first[:], wcol.ap()[0])

            brow_t = cpool.tile([_P, _NRW * 2], f32)
            nc.sync.dma_start(brow_t[:], brow.ap())
            bcol_t = cpool.tile([_P, _NE * _KT], f32)
            nc.sync.dma_start(bcol_t[:], bcol.ap())
            bfull_t = cpool.tile([_P, _NF * _KT], f32)
            nc.sync.dma_start(bfull_t[:], bfull.ap())

            # persistent landing area: per exchange, 8 rank slots + bias
            # slot 9 (pre-filled once; the reduce then covers the bias)
            _SL = (_M + 1) * _KT  # 144 floats per exchange
            landall = cpool.tile([_P, _NE * _SL], bf16)
            nc.vector.tensor_copy(
                landall[:].rearrange("p (j s) -> p j s", j=_NE)[:, :, _M * _KT:],
                bcol_t[:].rearrange("p (j t) -> p j t", j=_NE))

            v = vpool.tile([_P, 2 * _KT], bf16, tag="v")
            nc.sync.dma_start(v[:], x0.ap())

            def hi_lo_reduce(ps, n, tag):
                """PSUM [p, (n, h)] -> SBUF f32 [p, n] (hi+lo summed)."""
                sm = rpool.tile([_P, n], f32, tag=tag)
                nc.vector.tensor_reduce(
                    sm[:],
                    ps[:].rearrange("p (n h) -> p n h", h=2),
                    axis=mybir.AxisListType.X,
                    op=add,
                )
                return sm

            def silu_split(pre, n, tag):
                """f32 [p, n] -> bf16 [p, (h, n)] hi/lo pair of silu."""
                s32 = spool.tile([_P, n], f32, tag=tag + "32")
                nc.scalar.activation(s32[:], pre[:], silu)
                s = spool.tile([_P, 2 * n], bf16, tag=tag)
                nc.vector.tensor_copy(s[:, 0:n], s32[:])
                nc.vector.tensor_tensor(s[:, n:2 * n], s32[:], s[:, 0:n], sub)
                return s

            def row_layer(i6, v, w=None):
                """v [p,(h,t)] bf16 -> pre [p, 2] f32 (bias added)."""
                if w is None:
                    w = wpool.tile([_P, 2 * _KT * _P], bf16, tag="w")
                    nc.sync.dma_start(w[:], wrow.ap()[i6])
                ps = psrpool.tile([_P, 4], f32, tag="psr")
                for c in range(2):
                    for kt in range(_KT):
                        f = (kt * 2 + c) * _P
                        nc.tensor.matmul(
                            ps[:, 2 * c:2 * c + 2],
                            lhsT=w[:, f:f + _P],
                            rhs=v[:, kt:kt + 17:16],
                            start=(kt == 0),
                            stop=(kt == _KT - 1),
                        )
                sm = hi_lo_reduce(ps, 2, "rsum")
                pre = rpool.tile([_P, 2], f32, tag="prer")
                nc.vector.tensor_tensor(
                    pre[:], sm[:], brow_t[:, i6 * 2:i6 * 2 + 2], add)
                return pre

            for seg in range(_NE):
                # --- row layer ---
                pre = row_layer(seg, v, w=w_first if seg == 0 else None)
                s = silu_split(pre, 2, "s")

                # --- col layer: partial over all 2048 outputs ---
                if seg == 0:
                    w = wc_first
                else:
                    w = wpool.tile([_P, 2 * _KT * _P], bf16, tag="w")
                    nc.sync.dma_start(w[:], wcol.ap()[seg])
                pc = pscpool.tile([_P, 2 * _KT], f32, tag="psc")
                for mt in range(_KT):
                    for c in range(2):
                        f = (c * _KT + mt) * _P
                        nc.tensor.matmul(
                            pc[:, 2 * mt:2 * mt + 2],
                            lhsT=w[:, f:f + _P],
                            rhs=s[:, c:c + 3:2],
                            start=(c == 0),
                            stop=(c == 1),
                        )
                # bf16 reduce output feeds the exchange directly (the
                # partial is re-rounded to bf16 for the wire anyway)
                sp = spool.tile([_P, _KT], bf16, tag="spb")
                with nc.allow_low_precision("bf16 exchange partial"):
                    nc.vector.tensor_reduce(
                        sp[:],
                        pc[:].rearrange("p (n h) -> p n h", h=2),
                        axis=mybir.AxisListType.X,
                        op=add,
                    )

                # --- exchange: AllGather partials, reduce + bias + silu ---
                cc_in = dpool.tile([_P, _KT], bf16, tag="ccin")
                nc.sync.dma_start(cc_in[:], sp[:])
                cc_out = dpool.tile([_M, _N], bf16, tag="ccout")
                nc.gpsimd.collective_compute(
                    "AllGather",
                    mybir.AluOpType.bypass,
                    replica_groups=[list(range(_M))],
                    ins=[cc_in.opt()],
                    outs=[cc_out.opt()],
                )
                land = landall[:, seg * _SL:seg * _SL + _M * _KT]
                nc.sync.dma_start(
                    land.rearrange("p (r t) -> p r t", r=_M),
                    cc_out[:, :].rearrange("r (p t) -> p r t", p=_P))
                pre_v = rpool.tile([_P, _KT], f32, tag="prev")
                nc.vector.tensor_reduce(
                    pre_v[:],
                    landall[:, seg * _SL:(seg + 1) * _SL]
                    .rearrange("p (r t) -> p t r", r=_M + 1),
                    axis=mybir.AxisListType.X,
                    op=add,
                )
                v = silu_split(pre_v, _KT, "v")

                # --- full layer (replicated on every core) ---
                if seg in _FULL_AFTER:
                    fi = _FULL_AFTER.index(seg)
                    wf = wfhpool.tile([_P, _KT * _KT * _P], bf16, tag="wf")
                    # split the 4MB load into 4 chunks so a latency-critical
                    # exchange bounce DMA never queues behind more than ~6us
                    # of weight transfer on the SP DMA path
                    _ck = _KT * _KT * _P // 4
                    for _c in range(4):
                        nc.sync.dma_start(
                            wf[:, _c * _ck:(_c + 1) * _ck],
                            wfull.ap()[fi][:, _c * _ck:(_c + 1) * _ck])
                    pf = pscpool.tile([_P, 2 * _KT], f32, tag="psf")
                    for mt in range(_KT):
                        for kt in range(_KT):
                            f = (kt * _KT + mt) * _P
                            nc.tensor.matmul(
                                pf[:, 2 * mt:2 * mt + 2],
                                lhsT=wf[:, f:f + _P],
                                rhs=v[:, kt:kt + 17:16],
                                start=(kt == 0),
                                stop=(kt == _KT - 1),
                            )
                    sf = hi_lo_reduce(pf, _KT, "sf")
                    pre_f = rpool.tile([_P, _KT], f32, tag="pref")
                    nc.vector.tensor_tensor(
                        pre_f[:], sf[:],
                        bfull_t[:, fi * _KT:(fi + 1) * _KT], add)
                    v = silu_split(pre_f, _KT, "v")

            # --- last row layer (identity activation, f32 out) ---
            pre = row_layer(_NRW - 1, v)
            nc.sync.dma_start(out.ap(), pre[:])

    nc.compile()
    return nc


def _prep_inputs(x, W, b):
    import ml_dtypes
    bf16 = ml_dtypes.bfloat16
    W = np.ascontiguousarray(W, dtype=np.float32)
    b = np.ascontiguousarray(b, dtype=np.float32)
    x = np.ascontiguousarray(x, dtype=np.float32)

    # row weights: [i6][cc][p, (kt, c, m)], elem = W[i][cc*256+c*128+m, p*16+kt]
    wrow = np.empty((_M, _NRW, _P, 2 * _KT * _P), dtype=bf16)
    for n, i in enumerate(_ROW_IDS):
        Wi = W[i].reshape(_M, 2, _P, _P, _KT)       # [cc, c, m, p, t]
        Wi = Wi.transpose(0, 3, 4, 1, 2)            # [cc, p, t, c, m]
        wrow[:, n] = Wi.reshape(_M, _P, 2 * _KT * _P).astype(bf16)

    # col weights: [i5][cc][p, (c, mt, m)], elem = W[i][m*16+mt, cc*256+c*128+p]
    wcol = np.empty((_M, _NE, _P, 2 * _KT * _P), dtype=bf16)
    for n, i in enumerate(_COL_IDS):
        Wi = W[i].reshape(_P, _KT, _M, 2, _P)       # [m, mt, cc, c, p]
        Wi = Wi.transpose(2, 4, 3, 1, 0)            # [cc, p, c, mt, m]
        wcol[:, n] = Wi.reshape(_M, _P, 2 * _KT * _P).astype(bf16)

    # full weights (replicated): [i4][p, (kt, mt, m)], elem = W[i][m*16+mt, p*16+kt]
    wfull = np.zeros((max(_NF, 1), _P, _KT * _KT * _P), dtype=bf16)
    for n, i in enumerate(_FULL_IDS):
        Wi = W[i].reshape(_P, _KT, _P, _KT)         # [m, mt, p, kt]
        Wi = Wi.transpose(2, 3, 1, 0)               # [p, kt, mt, m]
        wfull[n] = Wi.reshape(_P, _KT * _KT * _P).astype(bf16)

    # row biases: [cc][p, (i6, c)] f32 = b[i][cc*256 + c*128 + p]
    brow = np.empty((_M, _P, _NRW * 2), dtype=np.float32)
    for n, i in enumerate(_ROW_IDS):
        bi = b[i].reshape(_M, 2, _P)                # [cc, c, p]
        brow[:, :, 2 * n:2 * n + 2] = bi.transpose(0, 2, 1)

    # col/full biases (replicated): [p, (idx, t)] = b[i][p*16+t], f32
    bcol = np.empty((_P, _NE * _KT), dtype=np.float32)
    for n, i in enumerate(_COL_IDS):
        bcol[:, n * _KT:(n + 1) * _KT] = b[i].reshape(_P, _KT)
    bfull = np.zeros((_P, max(_NF, 1) * _KT), dtype=np.float32)
    for n, i in enumerate(_FULL_IDS):
        bfull[:, n * _KT:(n + 1) * _KT] = b[i].reshape(_P, _KT)

    # x hi/lo split: [p, (h, t)] with value x[p*16+t]
    xm = x.reshape(_P, _KT)
    xhi = xm.astype(bf16)
    xlo = (xm - xhi.astype(np.float32)).astype(bf16)
    x0 = np.concatenate([xhi, xlo], axis=1)

    in_maps = []
    for c in range(_M):
        in_maps.append({
            "wrow": np.ascontiguousarray(wrow[c]),
            "wcol": np.ascontiguousarray(wcol[c]),
            "wfull": wfull,
            "brow": np.ascontiguousarray(brow[c]),
            "bcol": bcol,
            "bfull": bfull,
            "x0": x0,
        })
    return in_maps


def kernel(x, W, b, _trace=False):
    from concourse.bass_utils import run_bass_kernel_spmd

    key = "nc"
    if key not in _CACHE:
        _CACHE[key] = _build()
    nc = _CACHE[key]

    in_maps = _prep_inputs(x, W, b)
    res = run_bass_kernel_spmd(
        nc, in_maps, core_ids=list(range(_M)), trace=_trace)
    _CACHE["last_results"] = res
    # out[p, c] holds global neuron core*256 + c*128 + p
    return np.concatenate(
        [res.results[c]["out"].T.reshape(-1) for c in range(_M)])
